# revision 7
# baseline (speedup 1.0000x reference)
"""Trainium2 Bass kernel for nn_AffNet (affinity network).

Reference computation:
    X_emb = X @ W                               # [N, E]
    aff_h = (Z_h @ X_emb^T) / (|X_emb| |Z_h|)   # cosine, [H, N, N]
    aff   = max_h aff_h                          # [N, N]
    aff   = (aff + aff^T) / 2                    # symmetrize
    aff   = (aff + 1) / 2                        # [0, 1]
    aff   = aff ** beta

Device strategy (8 NeuronCores, output-row parallel):
  Each core computes 2 of the 16 block-rows of the POOLED (pre-symmetrize)
  affinity A = max_h(Zh_hat @ Xh_hat^T): 8 m-chunk rows x 16 col blocks of
  [128, 512] tiles. The device only pools 4 heads -> 2 half-pooled planes;
  the host finishes with A = max(plane0, plane1), then
  out = 0.25*(A + A^T) + 0.5 and ^beta.

  Rationale (cost model): TensorTensor ops allow at most one PSUM operand,
  the Pool engine supports no two-tensor elementwise op at all, and matmul
  PSUM output is fp32-only on TRN2. So PSUM evacuation (ScalarE copies +
  VectorE 1-PSUM maxes) is the hard floor (~262K free-elem units/core
  across Act+DVE). Shipping two bf16 planes per tile instead of one moves
  the final merge to the host, trading idle DMA bandwidth (~102us, under
  the ~145us engine floor) for the DVE L2 work.

  All operands stay resident in SBUF (Xhat^T replicated 16KB/partition,
  Zhat^T own rows 8KB/partition; one DMA each). Outputs accumulate into a
  [128, 16, 2, 512] ship-stripe per m-chunk row -> 8 output DMAs per core.

  Per tile: 4 matmuls (heads) into two independently rotating 2-bank
  PSUM pairs (apr: heads {0,1} -> ScalarE; dpr: heads {2,3} -> VectorE;
  bufs=2 each => all 8 banks). Static route mix balancing Act ~142us =
  DVE ~142us busy (cost model; ~92% occupancy):
    A2 (119 tiles): ScalarE fused-copies the apr pair to bf16 (e01);
        VectorE does ONE fused max(dpr, e01) [128,2,512] straight into
        the output segment (planes {max(h2,h0), max(h3,h1)}).
    A4H (9 tiles): ScalarE evacuates BOTH pairs (apr -> segment planes,
        heads {2,3} via a second apr-tag tile -> side buffer); the DVE
        skips the tile and the host maxes all 4 planes. Keeping the
        dpr rotation DVE-only avoids cross-engine burst stalls.
  The apr side (matmuls + ScalarE copy) is emitted LOOK=6 tiles ahead of
  the DVE side so ScalarE banks enough lead to absorb A4H bursts.
  Outputs ship as 4-column segments (per-column on the final row) so the
  DMA tail after the last max is short; inputs stream in finely chunked
  DMAs ordered so the first tile's operands land ~4us in.
"""

import numpy as np

N_NODES = 8192
N_FEATURES = 512
EMB = 128
N_HEADS = 4
EPS = 1e-6
N_CORES = 8
BLK = 512
N_BLK = N_NODES // BLK          # 16 col blocks
M_CHUNK = 128
BLOCKS_PER_CORE = 2             # block-rows per core
MROWS = BLOCKS_PER_CORE * (BLK // M_CHUNK)   # 8 m-chunk rows per core
ROWS_PER_CORE = BLOCKS_PER_CORE * BLK        # 1024

_CACHE = {}
LAST_RESULT = None


def _a4h_cols(q):
    """Columns of m-row q on the Act-only A4H route (8 of 128): ScalarE
    evacuates both PSUM pairs (2 planes to the segment, 2 to a side
    buffer), the DVE skips the tile, and the host maxes all 4 planes.
    Rebalances Act ~142us / DVE ~142us without a DVE-blocking burst."""
    return (7, 11) if q == 3 else (7,)


A4H_TILES = [(q, j) for q in range(MROWS) for j in _a4h_cols(q)]


def _split_multi_waits(nc, limit=1):
    """The walrus build in this environment encodes at most one semaphore
    wait per instruction ("Too many sync wait commands" otherwise), while
    Tile attaches several. Hoist extra waits onto same-engine NOPs inserted
    immediately before the instruction (waits still execute before it)."""
    import concourse.mybir as mybir

    for f in nc.m.functions:
        for bb in f.blocks:
            il = bb.instructions  # live list backing the block
            idx = 0
            while idx < len(il):
                inst = il[idx]
                si = inst.sync_info
                waits = list(si.on_wait) if si is not None and si.on_wait else []
                if len(waits) > limit:
                    ups = list(si.on_update) if si.on_update else []
                    inst.sync_info = mybir.SyncInfo(
                        on_wait=waits[:limit], on_update=ups
                    )
                    eng = nc.engines[inst.engine]
                    pos = idx
                    for j in range(limit, len(waits), limit):
                        nbi = eng.nop()
                        ninst = nbi.ins
                        # nop() appended itself to the current bb; detach it
                        removed = False
                        for f2 in nc.m.functions:
                            for bb2 in f2.blocks:
                                l2 = bb2.instructions
                                if l2 and l2[-1].name == ninst.name:
                                    l2.pop()
                                    removed = True
                                    break
                            if removed:
                                break
                        assert removed, "could not detach helper nop"
                        ninst.sync_info = mybir.SyncInfo(
                            on_wait=waits[j : j + limit], on_update=[]
                        )
                        il.insert(pos, ninst)
                        pos += 1
                        idx += 1
                idx += 1


def _build_program():
    import concourse.bass as bass
    import concourse.mybir as mybir
    import concourse.tile as tile

    nc = bass.Bass("TRN2", target_bir_lowering=False, debug=False)

    bf16 = mybir.dt.bfloat16
    f32 = mybir.dt.float32

    # Xhat^T full, [E, 16, 512]; Zhat^T own rows, [E, H, 1024]
    xt = nc.dram_tensor("xt", [EMB, N_BLK, BLK], bf16, kind="ExternalInput")
    zt = nc.dram_tensor("zt", [EMB, N_HEADS, ROWS_PER_CORE], bf16,
                        kind="ExternalInput")
    fp8 = mybir.dt.float8e4
    # three planes per tile (2 raw from ScalarE, 1 pooled from VectorE)
    aout = nc.dram_tensor("aout", [MROWS, M_CHUNK, N_BLK, 3, BLK], fp8,
                          kind="ExternalOutput")
    # extra planes for the A4H tiles: host maxes these in
    bout = nc.dram_tensor("bout", [len(A4H_TILES), M_CHUNK, 2, BLK], fp8,
                          kind="ExternalOutput")

    with tile.TileContext(nc) as tc:
        with (
            tc.tile_pool(name="weights", bufs=1) as wpool,
            tc.tile_pool(name="psum", bufs=1, space="PSUM") as ppool,
            tc.tile_pool(name="work", bufs=2) as spool,
        ):
            xt_s = wpool.tile([EMB, N_BLK, BLK], bf16, name="xt_s")
            zt_s = wpool.tile([EMB, N_HEADS, ROWS_PER_CORE], bf16,
                              name="zt_s")
            # Load order tuned for a fast first tile: the row-0 slices of
            # zt and the first xt block arrive within ~2us, rest streams.
            nc.sync.dma_start(out=zt_s[:, 0:2, 0:M_CHUNK],
                              in_=zt[:, 0:2, 0:M_CHUNK])
            nc.sync.dma_start(out=xt_s[:, 0:1], in_=xt[:, 0:1])
            nc.sync.dma_start(out=zt_s[:, 2:4, 0:M_CHUNK],
                              in_=zt[:, 2:4, 0:M_CHUNK])
            nc.sync.dma_start(out=xt_s[:, 1:3], in_=xt[:, 1:3])
            nc.sync.dma_start(out=xt_s[:, 3:6], in_=xt[:, 3:6])
            nc.sync.dma_start(out=zt_s[:, 0:2, M_CHUNK:],
                              in_=zt[:, 0:2, M_CHUNK:])
            nc.sync.dma_start(out=xt_s[:, 6:11], in_=xt[:, 6:11])
            nc.sync.dma_start(out=zt_s[:, 2:4, M_CHUNK:],
                              in_=zt[:, 2:4, M_CHUNK:])
            nc.sync.dma_start(out=xt_s[:, 11:16], in_=xt[:, 11:16])

            tiles = [(q, j) for q in range(MROWS) for j in range(N_BLK)]
            stripes = {}
            deferred = []

            def get_seg(q, j):
                key = (q, j // 4)
                if key not in stripes:
                    stripes[key] = spool.tile(
                        [M_CHUNK, 4, 3, BLK], fp8,
                        tag="seg", bufs=8, name=f"seg_{q}_{j // 4}")
                return stripes[key]

            def _flush_deferred():
                while deferred:
                    seg_, slot_, a_, b_ = deferred.pop(0)
                    nc.vector.tensor_max(seg_[:, slot_], a_, b_)

            def emit_apr_side(t):
                """Act-pair matmuls + ScalarE evacuation for tile t.
                Emitted one tile AHEAD of the DVE side so the e01 operand
                is ready a full tile before the DVE max needs it. A4H
                tiles evacuate straight into the output segment (slot 3)
                and skip the DVE entirely."""
                q, j = tiles[t]
                msl = slice(q * M_CHUNK, (q + 1) * M_CHUNK)
                rhs = xt_s[:, j]
                apr = ppool.tile([M_CHUNK, 2, BLK], f32, tag="apr",
                                 bufs=2, name=f"apr_{q}_{j}")
                nc.tensor.matmul(apr[:, 0], zt_s[:, 0, msl], rhs,
                                 start=True, stop=True)
                nc.tensor.matmul(apr[:, 1], zt_s[:, 1, msl], rhs,
                                 start=True, stop=True)
                nc.scalar.copy(get_seg(q, j)[:, j % 4, 0:2], apr)
                return None

            def emit_dpr_side(t, e01):
                q, j = tiles[t]
                msl = slice(q * M_CHUNK, (q + 1) * M_CHUNK)
                rhs = xt_s[:, j]
                seg = get_seg(q, j)
                if j in _a4h_cols(q):
                    # Act-only route: heads {2,3} land in a SECOND
                    # apr-tag tile (Act-paced rotation -- keeping the
                    # dpr rotation DVE-only) and ship via the side
                    # buffer; the idle Pool engine pads the pooled
                    # plane with -448 so the host can max all planes.
                    apr2 = ppool.tile([M_CHUNK, 2, BLK], f32, tag="apr",
                                      bufs=2, name=f"apr2_{q}_{j}")
                    nc.tensor.matmul(apr2[:, 0], zt_s[:, 2, msl], rhs,
                                     start=True, stop=True)
                    nc.tensor.matmul(apr2[:, 1], zt_s[:, 3, msl], rhs,
                                     start=True, stop=True)
                    bseg = spool.tile([M_CHUNK, 2, BLK], fp8, tag="bseg",
                                      bufs=2, name=f"bseg_{q}_{j}")
                    nc.scalar.copy(bseg, apr2)
                    nc.sync.dma_start(out=bout[A4H_TILES.index((q, j))],
                                      in_=bseg)
                    nc.gpsimd.memset(seg[:, j % 4, 2], -448.0)
                else:
                    dpr = ppool.tile([M_CHUNK, 2, BLK], f32, tag="dpr",
                                     bufs=2, name=f"dpr_{q}_{j}")
                    nc.tensor.matmul(dpr[:, 0], zt_s[:, 2, msl], rhs,
                                     start=True, stop=True)
                    nc.tensor.matmul(dpr[:, 1], zt_s[:, 3, msl], rhs,
                                     start=True, stop=True)
                    # Main route: DVE pools its pair independently
                    # (no ScalarE handoff at all)
                    nc.vector.tensor_reduce(
                        seg[:, j % 4, 2], dpr.transpose([0, 2, 1]),
                        axis=mybir.AxisListType.X, op=mybir.AluOpType.max)
                    _flush_deferred()
                if q == MROWS - 1 and j >= N_BLK - 8:
                    # tail: per-column shipping so the final DMA is short
                    nc.sync.dma_start(
                        out=aout[q, :, j:j + 1], in_=seg[:, j % 4:j % 4 + 1])
                elif j % 4 == 3:
                    if deferred:
                        _flush_deferred()
                    nc.sync.dma_start(
                        out=aout[q, :, j - 3:j + 1], in_=seg)

            # 3-tile apr lookahead: PE banks apr matmuls before each dpr
            # stall, so ScalarE can run far enough ahead to absorb the
            # A4H copy bursts without starving the DVE.
            LOOK = 16
            pend = [emit_apr_side(t) for t in range(min(LOOK, len(tiles)))]
            for t in range(len(tiles)):
                if t + LOOK < len(tiles):
                    pend.append(emit_apr_side(t + LOOK))
                emit_dpr_side(t, pend.pop(0))

    _split_multi_waits(nc)
    return nc


def kernel(X, W, Z, beta):
    global LAST_RESULT
    import ml_dtypes
    from concourse.bass_utils import run_bass_kernel_spmd

    X = np.asarray(X, dtype=np.float32)
    W = np.asarray(W, dtype=np.float32)
    Z = np.asarray(Z, dtype=np.float32)
    beta_f = float(np.asarray(beta))

    bf16 = ml_dtypes.bfloat16

    # Host: normalized, transposed bf16 operands
    X_emb = X @ W                                            # [N, E] fp32
    Xn = np.sqrt(np.sum(X_emb * X_emb, axis=-1))             # [N]
    Zn = np.sqrt(np.sum(Z * Z, axis=-1))                     # [H, N]
    Xh = X_emb / (Xn[:, None] + EPS)                         # [N, E]
    Zh = Z / (Zn[:, :, None] + EPS)                          # [H, N, E]
    XT = np.ascontiguousarray(Xh.T).astype(bf16)             # [E, N]
    xt_full = XT.reshape(EMB, N_BLK, BLK)

    if "nc" not in _CACHE:
        _CACHE["nc"] = _build_program()
    nc = _CACHE["nc"]

    in_maps = []
    for c in range(N_CORES):
        rows = slice(c * ROWS_PER_CORE, (c + 1) * ROWS_PER_CORE)
        # [E, H, 1024]
        zt_c = np.ascontiguousarray(
            Zh[:, rows, :].transpose(2, 0, 1)
        ).astype(bf16)
        in_maps.append({"xt": xt_full, "zt": zt_c})

    res = None
    for attempt in range(3):
        try:
            res = run_bass_kernel_spmd(nc, in_maps, list(range(N_CORES)))
            break
        except Exception:
            if attempt == 2:
                raise
    LAST_RESULT = res

    # Assemble pooled A: host merges the two shipped planes, plus the two
    # extra A4H planes for the j==7 blocks.
    A = np.empty((N_NODES, N_NODES), dtype=np.float32)
    for c in range(N_CORES):
        a_c = res.results[c]["aout"]  # [8, 128, 16, 3, 512] fp8
        b_c = res.results[c]["bout"]  # [n_a4h, 128, 2, 512] fp8
        planes = a_c.reshape(ROWS_PER_CORE, N_BLK, 3, BLK)
        rows = slice(c * ROWS_PER_CORE, (c + 1) * ROWS_PER_CORE)
        Ar = A[rows].reshape(ROWS_PER_CORE, N_BLK, BLK)
        np.max(planes.astype(np.float32), axis=2, out=Ar)
        for idx, (q, j) in enumerate(A4H_TILES):
            bmax = np.maximum(
                b_c[idx, :, 0].astype(np.float32),
                b_c[idx, :, 1].astype(np.float32),
            )
            rsl = slice(q * M_CHUNK, (q + 1) * M_CHUNK)
            np.maximum(Ar[rsl, j], bmax, out=Ar[rsl, j])

    out = np.empty_like(A)
    B = 1024
    nb = N_NODES // B
    for bi in range(nb):
        ri = slice(bi * B, (bi + 1) * B)
        for bj in range(bi, nb):
            cj = slice(bj * B, (bj + 1) * B)
            S = A[ri, cj] + A[cj, ri].T
            S *= np.float32(0.25)
            S += np.float32(0.5)
            out[ri, cj] = S
            if bj != bi:
                out[cj, ri] = S.T

    if beta_f != 1.0:
        out = np.power(out, beta_f, dtype=np.float32)
    return out
